# revision 17
# baseline (speedup 1.0000x reference)
"""Trainium2 Bass kernel for nn_MultiHeadSelfAttention (B=2, N=2048, C=1024, H=16).

Sharding: 8 cores = (batch b in {0,1}) x (head-group g in {0..3}); each core
computes 4 heads of one batch plus its partial output projection. The host
sums the 4 partial projections per batch and adds the bias constant
(v-bias and proj-bias folded together; k-bias is softmax-invariant and
dropped; q-bias applied on device).

Layouts (all transposed so no on-chip transposes needed):
  qT,kT [256,2048] = W_{q,k} @ x^T
  v     [2048,256] = x @ W_v^T       (bf16)
  S'_h  [kv,q] = K_h q_h^T           (row-tiled pairs, concurrent on PE)
  P'    = exp(S')                    (ACT, psum->sbuf, bf16)
  O^T_h [64,q] = V_h^T P'_h          (col-tiled pairs)
  D_h   [1,q]  = ones^T P'_h         (M=1 matmuls, col-tiled pair)

Schedule: the attention sweeps s=(n,hp) run as 8 pipelined slots per rep;
slot t interleaves, per kv chunk i, the QK+exp of sweep t with the AV +
denominator matmuls of sweep t-1 (slot 0 consumes the previous rep's last
sweep), so the Scalar engine (exp) never drains. The QKV projection of rep
r+1 and the y-projection units ride the slots as fine-grained extra matmuls,
with q/k/v and O buffers double-buffered by rep parity. The rep loop is a
hardware For_i over an unrolled parity pair; a prologue rep (internal lag)
fills the pipeline and an epilogue drains the last sweep.
"""
import sys
import os

sys.path.insert(0, "/opt/trn_rl_repo")

import numpy as np
import ml_dtypes

import concourse.bass as bass
import concourse.mybir as mybir
from concourse import bacc
from concourse.tile import TileContext
from concourse.bass_utils import run_bass_kernel_spmd

F32R = mybir.dt.float32r
F32 = mybir.dt.float32
BF16 = mybir.dt.bfloat16
MM_BF16 = os.environ.get("KMMDT", "bf16") == "bf16"
MMDT = BF16 if MM_BF16 else F32R
# P (=exp scores) and V in fp8: halves PE-side SBUF stream bandwidth for the
# AV/denominator column-pairs; quantization error averages out in O = P V / D
P8 = os.environ.get("KP8", "0") == "1"
PDT = mybir.dt.float8e4 if P8 else BF16
Exp = mybir.ActivationFunctionType.Exp

B, N, C, H = 2, 2048, 1024, 16
HD = C // H          # 64
SCALE = 1.0 / np.sqrt(HD).astype(np.float32)

NQ = N // 512        # 4 q-chunks of 512
NK = N // 128        # 16 kv-chunks of 128
NJ = C // 128        # 8 contraction chunks for projections


def build_nc():
    part = os.environ.get("KPART", "full")
    reps = int(os.environ.get("KREPS", "1"))
    nc = bacc.Bacc("TRN2", target_bir_lowering=False, debug=False, num_devices=8)

    xt_d = nc.dram_tensor("xt", [C, N], MMDT, kind="ExternalInput").ap()
    wqk_d = nc.dram_tensor("wqk", [128, NJ, 512], MMDT, kind="ExternalInput").ap()
    wv_d = nc.dram_tensor("wv", [128, NJ, 256], MMDT, kind="ExternalInput").ap()
    qb_d = nc.dram_tensor("qb", [128, 2], F32, kind="ExternalInput").ap()
    pw_d = nc.dram_tensor("pw", [128, 2, 1024], MMDT, kind="ExternalInput").ap()
    one_d = nc.dram_tensor("onec", [128, 1], PDT, kind="ExternalInput").ap()
    out_d = nc.dram_tensor("out", [N, C], F32, kind="ExternalOutput").ap()

    ilv = reps > 1  # interleave next-rep projection into the slots

    with TileContext(nc) as tc:
        with tc.tile_pool(name="const", bufs=1) as const, \
             tc.tile_pool(name="persist", bufs=1) as persist, \
             tc.tile_pool(name="xs", bufs=8) as xs, \
             tc.tile_pool(name="pts", bufs=28) as pts, \
             tc.tile_pool(name="dsbp", bufs=2) as dsbp, \
             tc.tile_pool(name="dbp", bufs=2) as dbp, \
             tc.tile_pool(name="rbp", bufs=2) as rbp, \
             tc.tile_pool(name="oup", bufs=2) as oup, \
             tc.tile_pool(name="yts", bufs=4) as yts, \
             tc.tile_pool(name="scr", bufs=1, space="DRAM") as scr, \
             tc.tile_pool(name="psA", bufs=2, space="PSUM") as psA, \
             tc.tile_pool(name="psB", bufs=2, space="PSUM") as psB, \
             tc.tile_pool(name="psC", bufs=2, space="PSUM") as psC:

            wqk_t = const.tile([128, NJ, 512], MMDT)
            wv_t = const.tile([128, NJ, 256], MMDT)
            qb_t = const.tile([128, 2], F32)
            pw_t = const.tile([128, 2, 1024], MMDT)
            ones_t = const.tile([128, 1], PDT)

            npar = 2 if ilv else 1
            q_p = [persist.tile([128, 2, N], MMDT, name=f"q_all{i_}")
                   for i_ in range(npar)]
            k_p = [persist.tile([128, 2, N], MMDT, name=f"k_all{i_}")
                   for i_ in range(npar)]
            v_p = [persist.tile([128, NK, 256], PDT, name=f"v_all{i_}")
                   for i_ in range(npar)]
            on_p = [persist.tile([128, 2, N], MMDT, name=f"on_all{i_}")
                    for i_ in range(npar)]
            # slot-7 exp output crosses the For_i back edge (consumed by the
            # next rep's slot 0), so it lives in a persistent ring, not a pool
            pt7 = persist.tile([128, NK, 1024], PDT, name="pt7")
            dscr = [scr.tile([2, NQ, 2, 512], F32, name=f"dscr{i_}")
                    for i_ in range(npar)]

            # constants load once
            nc.scalar.dma_start(out=qb_t, in_=qb_d)
            nc.scalar.dma_start(out=ones_t, in_=one_d)
            nc.scalar.dma_start(out=pw_t, in_=pw_d)
            for j in range(NJ):
                nc.scalar.dma_start(out=wqk_t[:, j, :], in_=wqk_d[:, j, :])
                nc.scalar.dma_start(out=wv_t[:, j, :], in_=wv_d[:, j, :])

            def proj_loads(n):
                xts = []
                for j in range(NJ):
                    xt_t = xs.tile([128, 512], MMDT, tag="xt")
                    nc.sync.dma_start(
                        out=xt_t,
                        in_=xt_d[128 * j:128 * (j + 1), 512 * n:512 * (n + 1)])
                    xts.append(xt_t)
                return xts

            def proj_round_thunks(n, r, xts, par):
                """Round r of chunk n: two accumulation chains (one PSUM bank
                each) as a list of single-MM thunks; eviction rides the last."""
                nsl = slice(512 * n, 512 * (n + 1))
                c = [psC.tile([128, 512], F32, tag="psC", name=f"pr{r}{_m}")
                     for _m in range(2)]
                thunks = []
                for j in range(NJ):
                    st, sp = (j == 0), (j == NJ - 1)
                    for m in range(2):
                        if r < 2:
                            base = 256 * r + 128 * m

                            def mm(j=j, m=m, base=base, st=st, sp=sp):
                                nc.tensor.matmul(
                                    c[m], lhsT=wqk_t[:, j, base:base + 128],
                                    rhs=xts[j], start=st, stop=sp)
                        else:
                            t_ = 2 * (r - 2) + m

                            def mm(j=j, m=m, t_=t_, st=st, sp=sp):
                                nc.tensor.matmul(
                                    c[m][:, 0:256],
                                    lhsT=xts[j][:, 128 * t_:128 * (t_ + 1)],
                                    rhs=wv_t[:, j, :], start=st, stop=sp)
                        thunks.append(mm)

                def evict():
                    if r == 0:
                        for m in range(2):
                            nc.vector.tensor_scalar_add(
                                out=q_p[par][:, m, nsl], in0=c[m],
                                scalar1=qb_t[:, m:m + 1])
                    elif r == 1:
                        for m in range(2):
                            nc.vector.tensor_copy(out=k_p[par][:, m, nsl],
                                                  in_=c[m])
                    else:
                        for m in range(2):
                            t_ = 2 * (r - 2) + m
                            nc.vector.tensor_copy(
                                out=v_p[par][:, 4 * n + t_, :],
                                in_=c[m][:, 0:256])
                thunks.append(evict)
                return thunks

            def py_unit_thunk(pyn, u, par):
                def run():
                    m = 4 * pyn + u // 2
                    nn = u % 2
                    py = psC.tile([128, 512], F32, tag="psC", name="py")
                    for hp in range(2):
                        nc.tensor.matmul(
                            py, lhsT=on_p[par][:, hp, 128 * m:128 * (m + 1)],
                            rhs=pw_t[:, hp, 512 * nn:512 * (nn + 1)],
                            start=(hp == 0), stop=(hp == 1))
                    yt = yts.tile([128, 512], F32, tag="yt")
                    nc.vector.tensor_copy(out=yt, in_=py)
                    nc.sync.dma_start(
                        out=out_d[128 * m:128 * (m + 1),
                                  512 * nn:512 * (nn + 1)],
                        in_=yt)
                return run

            def finish_sweep(pn, php, par, oe_ps, oo_ps):
                pnsl = slice(512 * pn, 512 * (pn + 1))
                ou = oup.tile([128, 512], F32, tag="ou")
                nc.vector.tensor_copy(out=ou[0:64, :], in_=oe_ps[0:64, :])
                nc.vector.tensor_copy(out=ou[64:128, :], in_=oo_ps[64:128, :])
                dsb = dsbp.tile([128, 512], F32, tag="dsb")
                nc.vector.tensor_copy(out=dsb[96:97, :], in_=oe_ps[96:97, :])
                nc.vector.tensor_copy(out=dsb[0:1, :], in_=oo_ps[0:1, :])
                nc.gpsimd.dma_start(out=dscr[par][php, pn, 0, :], in_=dsb[96:97, :])
                nc.gpsimd.dma_start(out=dscr[par][php, pn, 1, :], in_=dsb[0:1, :])
                db = dbp.tile([128, 512], F32, tag="db")
                nc.gpsimd.dma_start(
                    out=db[0:64, :],
                    in_=dscr[par][php, pn, 0:1, :].to_broadcast([64, 512]))
                nc.gpsimd.dma_start(
                    out=db[64:128, :],
                    in_=dscr[par][php, pn, 1:2, :].to_broadcast([64, 512]))
                rb = rbp.tile([128, 512], F32, tag="rb")
                nc.vector.reciprocal_approx_fast(out=rb, in_=db)
                nc.vector.tensor_mul(out=on_p[par][:, php, pnsl], in0=ou, in1=rb)

            def emit_av_group(prev, ppar, ptp, i, oe_ps, oo_ps):
                pn, php = prev
                st, sp = (i == 0), (i == NK - 1)
                nc.tensor.matmul(oe_ps[0:64, :],
                                 lhsT=v_p[ppar][:, i, 128 * php:128 * php + 64],
                                 rhs=ptp[:, 0:512], start=st, stop=sp,
                                 tile_position=(0, 0))
                nc.tensor.matmul(oo_ps[64:128, :],
                                 lhsT=v_p[ppar][:, i, 128 * php + 64:128 * (php + 1)],
                                 rhs=ptp[:, 512:1024], start=st, stop=sp,
                                 tile_position=(0, 64))
                nc.tensor.matmul(oe_ps[96:97, :], lhsT=ones_t,
                                 rhs=ptp[:, 0:512], start=st, stop=sp,
                                 tile_position=(0, 96))
                nc.tensor.matmul(oo_ps[0:1, :], lhsT=ones_t,
                                 rhs=ptp[:, 512:1024], start=st, stop=sp,
                                 tile_position=(0, 0))

            def emit_slots(p, pt_wrap, wrap_prev):
                """8 pipelined slots of one rep (parity p).

                pt_wrap/wrap_prev: pt list + (sweep, parity) whose AV runs in
                slot 0 (the previous rep's sweep 7), or None for internal lag
                (prologue: slot 0 has no AV).
                Returns the pt list + sweep of slot 7 (for the next rep/drain).
                """
                pt_prev, prev = pt_wrap, wrap_prev
                for t in range(8):
                    cur = (t // 2, t % 2)
                    n, hp = cur
                    nsl = slice(512 * n, 512 * (n + 1))
                    # extras for this slot
                    extras = []
                    if ilv:
                        ch = t // 2   # proj chunk rides slots 2ch, 2ch+1
                        if t % 2 == 0:
                            xts_c = proj_loads(ch)
                            emit_slots.xts = xts_c
                            for r in range(3):
                                extras += proj_round_thunks(ch, r, xts_c, 1 - p)
                        else:
                            extras += proj_round_thunks(ch, 3, emit_slots.xts,
                                                        1 - p)
                    if t == 1 and wrap_prev is not None:
                        for u in range(8):
                            extras.append(py_unit_thunk(3, u, wrap_prev[2]))
                    if t in (3, 5, 7):
                        for u in range(8):
                            extras.append(py_unit_thunk((t - 3) // 2, u, p))
                    ne = len(extras)

                    pt_cur = []
                    oe_ps = oo_ps = None
                    for i in range(16):
                        isl = slice(128 * i, 128 * (i + 1))
                        s2 = psA.tile([128, 1024], F32, tag="psA", name="s2")
                        nc.tensor.matmul(s2[:, 0:512],
                                         lhsT=k_p[p][0:64, hp, isl],
                                         rhs=q_p[p][0:64, hp, nsl],
                                         start=True, stop=True)
                        nc.tensor.matmul(s2[:, 512:1024],
                                         lhsT=k_p[p][64:128, hp, isl],
                                         rhs=q_p[p][64:128, hp, nsl],
                                         start=True, stop=True)
                        if t == 7:
                            pt = pt7[:, i, :]
                        else:
                            pt = pts.tile([128, 1024], PDT, tag="pt")
                        nc.scalar.activation(out=pt, in_=s2, func=Exp)
                        pt_cur.append(pt)
                        if prev is not None:
                            if i == 0:
                                oe_ps = psB.tile([128, 512], F32, tag="psB",
                                                 name="oe_ps")
                                oo_ps = psB.tile([128, 512], F32, tag="psB",
                                                 name="oo_ps")
                            pw_par = prev[2]
                            emit_av_group(prev[:2], pw_par, pt_prev[i], i,
                                          oe_ps, oo_ps)
                        for e in extras[i * ne // 16:(i + 1) * ne // 16]:
                            e()
                    if prev is not None:
                        finish_sweep(prev[0], prev[1], prev[2], oe_ps, oo_ps)
                    pt_prev = pt_cur
                    prev = (cur[0], cur[1], p)
                return pt_prev, prev

            def emit_drain(pt_prev, prev):
                """AV + normalize + y-projection of the final sweep."""
                oe_ps = psB.tile([128, 512], F32, tag="psB", name="oe_ps")
                oo_ps = psB.tile([128, 512], F32, tag="psB", name="oo_ps")
                for i in range(16):
                    emit_av_group(prev[:2], prev[2], pt_prev[i], i, oe_ps, oo_ps)
                finish_sweep(prev[0], prev[1], prev[2], oe_ps, oo_ps)
                for u in range(8):
                    py_unit_thunk(3, u, prev[2])()

            # ---- prologue: rep 0 with a plain projection phase ----
            for n_ in range(NQ):
                xts0 = proj_loads(n_)
                for r_ in range(4):
                    for th in proj_round_thunks(n_, r_, xts0, 0):
                        th()
            pt_w, prev_w = emit_slots(0, None, None)

            # ---- steady-state bodies ----
            # unroll 4 bodies per For_i iteration: the loop back edge carries
            # an all-engine barrier (~15us), so amortize it over more reps
            nb = reps - 1
            U = 4

            def body(p):
                nonlocal pt_w, prev_w
                pt_w, prev_w = emit_slots(p, pt_w, prev_w)

            if nb >= U:
                with tc.For_i(0, nb // U, 1,
                              hint_engines=(mybir.EngineType.PE,
                                            mybir.EngineType.SP)):
                    for u_ in range(U):
                        body(1 - u_ % 2)
            for u_ in range(nb % U):
                body(1 - u_ % 2)

            # ---- epilogue: drain last sweep ----
            emit_drain(pt_w, prev_w)

            if part == "proj":
                for z in range(4):
                    nc.gpsimd.dma_start(
                        out=out_d[128 * z:128 * (z + 1), :],
                        in_=q_p[0][:, z // 2, 1024 * (z % 2):1024 * (z % 2 + 1)])
                    nc.gpsimd.dma_start(
                        out=out_d[128 * (4 + z):128 * (5 + z), :],
                        in_=k_p[0][:, z // 2, 1024 * (z % 2):1024 * (z % 2 + 1)])
                    vf = yts.tile([128, 4, 256], F32, tag="vf")
                    nc.vector.tensor_copy(out=vf, in_=v_p[0][:, 4 * z:4 * z + 4, :])
                    nc.sync.dma_start(
                        out=out_d[128 * (8 + z):128 * (9 + z), :].rearrange(
                            "p (a b) -> p a b", a=4),
                        in_=vf)
            elif part == "attn":
                for z in range(4):
                    onf = yts.tile([128, 1024], F32, tag="onf")
                    nc.vector.tensor_copy(
                        out=onf,
                        in_=on_p[0][:, z // 2, 1024 * (z % 2):1024 * (z % 2 + 1)])
                    nc.sync.dma_start(
                        out=out_d[128 * z:128 * (z + 1), :], in_=onf)

    nc.finalize()
    return nc


_NC = None


def _get_nc():
    global _NC
    if _NC is None:
        _NC = build_nc()
    return _NC


def make_in_maps(x, qkv_w, qkv_b, proj_w):
    """Host-side shard prep. Core c = 4*b + g handles batch b, heads 4g..4g+3."""
    x = np.asarray(x, np.float32)
    qkv_w = np.asarray(qkv_w, np.float32)
    qkv_b = np.asarray(qkv_b, np.float32)
    proj_w = np.asarray(proj_w, np.float32)
    in_maps = []
    onec = np.ones((128, 1), dtype=ml_dtypes.float8_e4m3 if P8
                   else ml_dtypes.bfloat16)
    for c in range(8):
        b, g = divmod(c, 4)
        hs = g * 4 * HD  # 256-wide feature slice for this core's heads
        xt = np.ascontiguousarray(x[b].T)                       # [C, N]
        wq = qkv_w[hs:hs + 256, :] * SCALE                      # pre-scaled q
        wk = qkv_w[C + hs:C + hs + 256, :]
        wqkT = np.ascontiguousarray(np.concatenate([wq, wk], 0).T)   # [C, 512]
        wqk = np.ascontiguousarray(wqkT.reshape(NJ, 128, 512).transpose(1, 0, 2))
        wvT = np.ascontiguousarray(qkv_w[2 * C + hs:2 * C + hs + 256, :].T)
        wv = np.ascontiguousarray(wvT.reshape(NJ, 128, 256).transpose(1, 0, 2))
        qb = np.ascontiguousarray((qkv_b[hs:hs + 256] * SCALE).reshape(2, 128).T)
        pwT = np.ascontiguousarray(proj_w[:, hs:hs + 256].T)    # [256, C]
        pw = np.ascontiguousarray(pwT.reshape(2, 128, 1024).transpose(1, 0, 2))
        if MM_BF16:
            bf = ml_dtypes.bfloat16
            xt, wqk, wv, pw = (a.astype(bf) for a in (xt, wqk, wv, pw))
        in_maps.append({"xt": xt, "wqk": wqk, "wv": wv, "qb": qb, "pw": pw,
                        "onec": onec})
    return in_maps


def unshard(results, qkv_b, proj_w, proj_b):
    cvec = (np.asarray(qkv_b, np.float32)[2 * C:] @ np.asarray(proj_w, np.float32).T
            + np.asarray(proj_b, np.float32))
    y = np.empty((B, N, C), np.float32)
    for b in range(B):
        acc = results[4 * b]["out"].copy()
        for g in range(1, 4):
            acc += results[4 * b + g]["out"]
        y[b] = acc + cvec[None, :]
    return y


def kernel(x, qkv_w, qkv_b, proj_w, proj_b):
    nc = _get_nc()
    in_maps = make_in_maps(x, qkv_w, qkv_b, proj_w)
    res = run_bass_kernel_spmd(nc, in_maps, core_ids=list(range(8)))
    return unshard(res.results, qkv_b, proj_w, proj_b)


# revision 18
# speedup vs baseline: 1.1017x; 1.1017x over previous
"""Trainium2 Bass kernel for nn_MultiHeadSelfAttention (B=2, N=2048, C=1024, H=16).

Sharding: 8 cores = (batch b in {0,1}) x (head-group g in {0..3}); each core
computes 4 heads of one batch plus its partial output projection. The host
sums the 4 partial projections per batch and adds the bias constant
(v-bias and proj-bias folded together; k-bias is softmax-invariant and
dropped; q-bias applied on device).

Layouts (all transposed so no on-chip transposes needed):
  qT,kT [256,2048] = W_{q,k} @ x^T
  v     [2048,256] = x @ W_v^T       (bf16)
  S'_h  [kv,q] = K_h q_h^T           (row-tiled pairs, concurrent on PE)
  P'    = exp(S')                    (ACT, psum->sbuf, bf16)
  O^T_h [64,q] = V_h^T P'_h          (col-tiled pairs)
  D_h   [1,q]  = ones^T P'_h         (M=1 matmuls, col-tiled pair)

Schedule: the attention sweeps s=(n,hp) run as 8 pipelined slots per rep;
slot t interleaves, per kv chunk i, the QK+exp of sweep t with the AV +
denominator matmuls of sweep t-1 (slot 0 consumes the previous rep's last
sweep), so the Scalar engine (exp) never drains. The QKV projection of rep
r+1 and the y-projection units ride the slots as fine-grained extra matmuls,
with q/k/v and O buffers double-buffered by rep parity. The rep loop is a
hardware For_i over an unrolled parity pair; a prologue rep (internal lag)
fills the pipeline and an epilogue drains the last sweep.
"""
import sys
import os

sys.path.insert(0, "/opt/trn_rl_repo")

import numpy as np
import ml_dtypes

import concourse.bass as bass
import concourse.mybir as mybir
from concourse import bacc
from concourse.tile import TileContext
from concourse.bass_utils import run_bass_kernel_spmd

F32R = mybir.dt.float32r
F32 = mybir.dt.float32
BF16 = mybir.dt.bfloat16
MM_BF16 = os.environ.get("KMMDT", "bf16") == "bf16"
MMDT = BF16 if MM_BF16 else F32R
# P (=exp scores) and V in fp8: halves PE-side SBUF stream bandwidth for the
# AV/denominator column-pairs; quantization error averages out in O = P V / D
P8 = os.environ.get("KP8", "0") == "1"
PDT = mybir.dt.float8e4 if P8 else BF16
Exp = mybir.ActivationFunctionType.Exp

B, N, C, H = 2, 2048, 1024, 16
HD = C // H          # 64
SCALE = 1.0 / np.sqrt(HD).astype(np.float32)

NQ = N // 512        # 4 q-chunks of 512
NK = N // 128        # 16 kv-chunks of 128
NJ = C // 128        # 8 contraction chunks for projections


def build_nc():
    part = os.environ.get("KPART", "full")
    reps = int(os.environ.get("KREPS", "1"))
    nc = bacc.Bacc("TRN2", target_bir_lowering=False, debug=False, num_devices=8)

    xt_d = nc.dram_tensor("xt", [C, N], MMDT, kind="ExternalInput").ap()
    wqk_d = nc.dram_tensor("wqk", [128, NJ, 512], MMDT, kind="ExternalInput").ap()
    wv_d = nc.dram_tensor("wv", [128, NJ, 256], MMDT, kind="ExternalInput").ap()
    qb_d = nc.dram_tensor("qb", [128, 2], F32, kind="ExternalInput").ap()
    pw_d = nc.dram_tensor("pw", [128, 2, 1024], MMDT, kind="ExternalInput").ap()
    one_d = nc.dram_tensor("onec", [128, 1], PDT, kind="ExternalInput").ap()
    out_d = nc.dram_tensor("out", [N, C], F32, kind="ExternalOutput").ap()

    ilv = reps > 1  # interleave next-rep projection into the slots

    with TileContext(nc) as tc:
        with tc.tile_pool(name="const", bufs=1) as const, \
             tc.tile_pool(name="persist", bufs=1) as persist, \
             tc.tile_pool(name="xs", bufs=8) as xs, \
             tc.tile_pool(name="pts", bufs=28) as pts, \
             tc.tile_pool(name="dsbp", bufs=2) as dsbp, \
             tc.tile_pool(name="dbp", bufs=2) as dbp, \
             tc.tile_pool(name="rbp", bufs=2) as rbp, \
             tc.tile_pool(name="oup", bufs=2) as oup, \
             tc.tile_pool(name="yts", bufs=4) as yts, \
             tc.tile_pool(name="scr", bufs=1, space="DRAM") as scr, \
             tc.tile_pool(name="psA", bufs=2, space="PSUM") as psA, \
             tc.tile_pool(name="psB", bufs=2, space="PSUM") as psB, \
             tc.tile_pool(name="psC", bufs=2, space="PSUM") as psC:

            wqk_t = const.tile([128, NJ, 512], MMDT)
            wv_t = const.tile([128, NJ, 256], MMDT)
            qb_t = const.tile([128, 2], F32)
            pw_t = const.tile([128, 2, 1024], MMDT)
            ones_t = const.tile([128, 1], PDT)

            npar = 2 if ilv else 1
            q_p = [persist.tile([128, 2, N], MMDT, name=f"q_all{i_}")
                   for i_ in range(npar)]
            k_p = [persist.tile([128, 2, N], MMDT, name=f"k_all{i_}")
                   for i_ in range(npar)]
            v_p = [persist.tile([128, NK, 256], PDT, name=f"v_all{i_}")
                   for i_ in range(npar)]
            on_p = [persist.tile([128, 2, N], MMDT, name=f"on_all{i_}")
                    for i_ in range(npar)]
            # slot-7 exp output crosses the For_i back edge (consumed by the
            # next rep's slot 0), so it lives in a persistent ring, not a pool
            pt7 = persist.tile([128, NK, 1024], PDT, name="pt7")
            dscr = [scr.tile([2, NQ, 2, 512], F32, name=f"dscr{i_}")
                    for i_ in range(npar)]

            # constants load once
            nc.scalar.dma_start(out=qb_t, in_=qb_d)
            nc.scalar.dma_start(out=ones_t, in_=one_d)
            nc.scalar.dma_start(out=pw_t, in_=pw_d)
            for j in range(NJ):
                nc.scalar.dma_start(out=wqk_t[:, j, :], in_=wqk_d[:, j, :])
                nc.scalar.dma_start(out=wv_t[:, j, :], in_=wv_d[:, j, :])

            def proj_loads(n):
                xts = []
                for j in range(NJ):
                    xt_t = xs.tile([128, 512], MMDT, tag="xt")
                    nc.sync.dma_start(
                        out=xt_t,
                        in_=xt_d[128 * j:128 * (j + 1), 512 * n:512 * (n + 1)])
                    xts.append(xt_t)
                return xts

            def proj_round_thunks(n, r, xts, par):
                """Round r of chunk n: two accumulation chains (one PSUM bank
                each) as a list of single-MM thunks; eviction rides the last."""
                nsl = slice(512 * n, 512 * (n + 1))
                c = [psC.tile([128, 512], F32, tag="psC", name=f"pr{r}{_m}")
                     for _m in range(2)]
                thunks = []
                for j in range(NJ):
                    st, sp = (j == 0), (j == NJ - 1)
                    for m in range(2):
                        if r < 2:
                            base = 256 * r + 128 * m

                            def mm(j=j, m=m, base=base, st=st, sp=sp):
                                nc.tensor.matmul(
                                    c[m], lhsT=wqk_t[:, j, base:base + 128],
                                    rhs=xts[j], start=st, stop=sp)
                        else:
                            t_ = 2 * (r - 2) + m

                            def mm(j=j, m=m, t_=t_, st=st, sp=sp):
                                nc.tensor.matmul(
                                    c[m][:, 0:256],
                                    lhsT=xts[j][:, 128 * t_:128 * (t_ + 1)],
                                    rhs=wv_t[:, j, :], start=st, stop=sp)
                        thunks.append(mm)

                def evict():
                    if r == 0:
                        for m in range(2):
                            nc.vector.tensor_scalar_add(
                                out=q_p[par][:, m, nsl], in0=c[m],
                                scalar1=qb_t[:, m:m + 1])
                    elif r == 1:
                        for m in range(2):
                            nc.vector.tensor_copy(out=k_p[par][:, m, nsl],
                                                  in_=c[m])
                    else:
                        for m in range(2):
                            t_ = 2 * (r - 2) + m
                            nc.vector.tensor_copy(
                                out=v_p[par][:, 4 * n + t_, :],
                                in_=c[m][:, 0:256])
                thunks.append(evict)
                return thunks

            def py_unit_thunk(pyn, u, par):
                def run():
                    m = 4 * pyn + u // 2
                    nn = u % 2
                    py = psC.tile([128, 512], F32, tag="psC", name="py")
                    for hp in range(2):
                        nc.tensor.matmul(
                            py, lhsT=on_p[par][:, hp, 128 * m:128 * (m + 1)],
                            rhs=pw_t[:, hp, 512 * nn:512 * (nn + 1)],
                            start=(hp == 0), stop=(hp == 1))
                    yt = yts.tile([128, 512], F32, tag="yt")
                    nc.vector.tensor_copy(out=yt, in_=py)
                    nc.sync.dma_start(
                        out=out_d[128 * m:128 * (m + 1),
                                  512 * nn:512 * (nn + 1)],
                        in_=yt)
                return run

            def finish_sweep(pn, php, par, oe_ps, oo_ps):
                pnsl = slice(512 * pn, 512 * (pn + 1))
                ou = oup.tile([128, 512], F32, tag="ou")
                nc.vector.tensor_copy(out=ou[0:64, :], in_=oe_ps[0:64, :])
                nc.vector.tensor_copy(out=ou[64:128, :], in_=oo_ps[64:128, :])
                dsb = dsbp.tile([128, 512], F32, tag="dsb")
                nc.vector.tensor_copy(out=dsb[96:97, :], in_=oe_ps[96:97, :])
                nc.vector.tensor_copy(out=dsb[0:1, :], in_=oo_ps[0:1, :])
                nc.sync.dma_start(out=dscr[par][php, pn, 0, :], in_=dsb[96:97, :])
                nc.sync.dma_start(out=dscr[par][php, pn, 1, :], in_=dsb[0:1, :])
                db = dbp.tile([128, 512], F32, tag="db")
                nc.sync.dma_start(
                    out=db[0:64, :],
                    in_=dscr[par][php, pn, 0:1, :].to_broadcast([64, 512]))
                nc.sync.dma_start(
                    out=db[64:128, :],
                    in_=dscr[par][php, pn, 1:2, :].to_broadcast([64, 512]))
                rb = rbp.tile([128, 512], F32, tag="rb")
                nc.vector.reciprocal_approx_fast(out=rb, in_=db)
                nc.vector.tensor_mul(out=on_p[par][:, php, pnsl], in0=ou, in1=rb)

            def emit_av_group(prev, ppar, ptp, i, oe_ps, oo_ps):
                pn, php = prev
                st, sp = (i == 0), (i == NK - 1)
                nc.tensor.matmul(oe_ps[0:64, :],
                                 lhsT=v_p[ppar][:, i, 128 * php:128 * php + 64],
                                 rhs=ptp[:, 0:512], start=st, stop=sp,
                                 tile_position=(0, 0))
                nc.tensor.matmul(oo_ps[64:128, :],
                                 lhsT=v_p[ppar][:, i, 128 * php + 64:128 * (php + 1)],
                                 rhs=ptp[:, 512:1024], start=st, stop=sp,
                                 tile_position=(0, 64))
                nc.tensor.matmul(oe_ps[96:97, :], lhsT=ones_t,
                                 rhs=ptp[:, 0:512], start=st, stop=sp,
                                 tile_position=(0, 96))
                nc.tensor.matmul(oo_ps[0:1, :], lhsT=ones_t,
                                 rhs=ptp[:, 512:1024], start=st, stop=sp,
                                 tile_position=(0, 0))

            def emit_slots(p, pt_wrap, wrap_prev):
                """8 pipelined slots of one rep (parity p).

                pt_wrap/wrap_prev: pt list + (sweep, parity) whose AV runs in
                slot 0 (the previous rep's sweep 7), or None for internal lag
                (prologue: slot 0 has no AV).
                Returns the pt list + sweep of slot 7 (for the next rep/drain).
                """
                pt_prev, prev = pt_wrap, wrap_prev
                for t in range(8):
                    cur = (t // 2, t % 2)
                    n, hp = cur
                    nsl = slice(512 * n, 512 * (n + 1))
                    # extras for this slot
                    extras = []
                    if ilv:
                        ch = t // 2   # proj chunk rides slots 2ch, 2ch+1
                        if t % 2 == 0:
                            xts_c = proj_loads(ch)
                            emit_slots.xts = xts_c
                            for r in range(3):
                                extras += proj_round_thunks(ch, r, xts_c, 1 - p)
                        else:
                            extras += proj_round_thunks(ch, 3, emit_slots.xts,
                                                        1 - p)
                    if t == 1 and wrap_prev is not None:
                        for u in range(8):
                            extras.append(py_unit_thunk(3, u, wrap_prev[2]))
                    if t in (3, 5, 7):
                        for u in range(8):
                            extras.append(py_unit_thunk((t - 3) // 2, u, p))
                    ne = len(extras)

                    pt_cur = []
                    oe_ps = oo_ps = None
                    for i in range(16):
                        isl = slice(128 * i, 128 * (i + 1))
                        s2 = psA.tile([128, 1024], F32, tag="psA", name="s2")
                        nc.tensor.matmul(s2[:, 0:512],
                                         lhsT=k_p[p][0:64, hp, isl],
                                         rhs=q_p[p][0:64, hp, nsl],
                                         start=True, stop=True)
                        nc.tensor.matmul(s2[:, 512:1024],
                                         lhsT=k_p[p][64:128, hp, isl],
                                         rhs=q_p[p][64:128, hp, nsl],
                                         start=True, stop=True)
                        if t == 7:
                            pt = pt7[:, i, :]
                        else:
                            pt = pts.tile([128, 1024], PDT, tag="pt")
                        nc.scalar.activation(out=pt, in_=s2, func=Exp)
                        pt_cur.append(pt)
                        if prev is not None:
                            if i == 0:
                                oe_ps = psB.tile([128, 512], F32, tag="psB",
                                                 name="oe_ps")
                                oo_ps = psB.tile([128, 512], F32, tag="psB",
                                                 name="oo_ps")
                            pw_par = prev[2]
                            emit_av_group(prev[:2], pw_par, pt_prev[i], i,
                                          oe_ps, oo_ps)
                        for e in extras[i * ne // 16:(i + 1) * ne // 16]:
                            e()
                    if prev is not None:
                        finish_sweep(prev[0], prev[1], prev[2], oe_ps, oo_ps)
                    pt_prev = pt_cur
                    prev = (cur[0], cur[1], p)
                return pt_prev, prev

            def emit_drain(pt_prev, prev):
                """AV + normalize + y-projection of the final sweep."""
                oe_ps = psB.tile([128, 512], F32, tag="psB", name="oe_ps")
                oo_ps = psB.tile([128, 512], F32, tag="psB", name="oo_ps")
                for i in range(16):
                    emit_av_group(prev[:2], prev[2], pt_prev[i], i, oe_ps, oo_ps)
                finish_sweep(prev[0], prev[1], prev[2], oe_ps, oo_ps)
                for u in range(8):
                    py_unit_thunk(3, u, prev[2])()

            # ---- prologue: rep 0 with a plain projection phase ----
            for n_ in range(NQ):
                xts0 = proj_loads(n_)
                for r_ in range(4):
                    for th in proj_round_thunks(n_, r_, xts0, 0):
                        th()
            pt_w, prev_w = emit_slots(0, None, None)

            # ---- steady-state bodies ----
            # unroll 4 bodies per For_i iteration: the loop back edge carries
            # an all-engine barrier (~15us), so amortize it over more reps
            nb = reps - 1
            U = 4

            def body(p):
                nonlocal pt_w, prev_w
                pt_w, prev_w = emit_slots(p, pt_w, prev_w)

            if nb >= U:
                with tc.For_i(0, nb // U, 1,
                              hint_engines=(mybir.EngineType.PE,
                                            mybir.EngineType.SP)):
                    for u_ in range(U):
                        body(1 - u_ % 2)
            for u_ in range(nb % U):
                body(1 - u_ % 2)

            # ---- epilogue: drain last sweep ----
            emit_drain(pt_w, prev_w)

            if part == "proj":
                for z in range(4):
                    nc.gpsimd.dma_start(
                        out=out_d[128 * z:128 * (z + 1), :],
                        in_=q_p[0][:, z // 2, 1024 * (z % 2):1024 * (z % 2 + 1)])
                    nc.gpsimd.dma_start(
                        out=out_d[128 * (4 + z):128 * (5 + z), :],
                        in_=k_p[0][:, z // 2, 1024 * (z % 2):1024 * (z % 2 + 1)])
                    vf = yts.tile([128, 4, 256], F32, tag="vf")
                    nc.vector.tensor_copy(out=vf, in_=v_p[0][:, 4 * z:4 * z + 4, :])
                    nc.sync.dma_start(
                        out=out_d[128 * (8 + z):128 * (9 + z), :].rearrange(
                            "p (a b) -> p a b", a=4),
                        in_=vf)
            elif part == "attn":
                for z in range(4):
                    onf = yts.tile([128, 1024], F32, tag="onf")
                    nc.vector.tensor_copy(
                        out=onf,
                        in_=on_p[0][:, z // 2, 1024 * (z % 2):1024 * (z % 2 + 1)])
                    nc.sync.dma_start(
                        out=out_d[128 * z:128 * (z + 1), :], in_=onf)

    nc.finalize()
    return nc


_NC = None


def _get_nc():
    global _NC
    if _NC is None:
        _NC = build_nc()
    return _NC


def make_in_maps(x, qkv_w, qkv_b, proj_w):
    """Host-side shard prep. Core c = 4*b + g handles batch b, heads 4g..4g+3."""
    x = np.asarray(x, np.float32)
    qkv_w = np.asarray(qkv_w, np.float32)
    qkv_b = np.asarray(qkv_b, np.float32)
    proj_w = np.asarray(proj_w, np.float32)
    in_maps = []
    onec = np.ones((128, 1), dtype=ml_dtypes.float8_e4m3 if P8
                   else ml_dtypes.bfloat16)
    for c in range(8):
        b, g = divmod(c, 4)
        hs = g * 4 * HD  # 256-wide feature slice for this core's heads
        xt = np.ascontiguousarray(x[b].T)                       # [C, N]
        wq = qkv_w[hs:hs + 256, :] * SCALE                      # pre-scaled q
        wk = qkv_w[C + hs:C + hs + 256, :]
        wqkT = np.ascontiguousarray(np.concatenate([wq, wk], 0).T)   # [C, 512]
        wqk = np.ascontiguousarray(wqkT.reshape(NJ, 128, 512).transpose(1, 0, 2))
        wvT = np.ascontiguousarray(qkv_w[2 * C + hs:2 * C + hs + 256, :].T)
        wv = np.ascontiguousarray(wvT.reshape(NJ, 128, 256).transpose(1, 0, 2))
        qb = np.ascontiguousarray((qkv_b[hs:hs + 256] * SCALE).reshape(2, 128).T)
        pwT = np.ascontiguousarray(proj_w[:, hs:hs + 256].T)    # [256, C]
        pw = np.ascontiguousarray(pwT.reshape(2, 128, 1024).transpose(1, 0, 2))
        if MM_BF16:
            bf = ml_dtypes.bfloat16
            xt, wqk, wv, pw = (a.astype(bf) for a in (xt, wqk, wv, pw))
        in_maps.append({"xt": xt, "wqk": wqk, "wv": wv, "qb": qb, "pw": pw,
                        "onec": onec})
    return in_maps


def unshard(results, qkv_b, proj_w, proj_b):
    cvec = (np.asarray(qkv_b, np.float32)[2 * C:] @ np.asarray(proj_w, np.float32).T
            + np.asarray(proj_b, np.float32))
    y = np.empty((B, N, C), np.float32)
    for b in range(B):
        acc = results[4 * b]["out"].copy()
        for g in range(1, 4):
            acc += results[4 * b + g]["out"]
        y[b] = acc + cvec[None, :]
    return y


def kernel(x, qkv_w, qkv_b, proj_w, proj_b):
    nc = _get_nc()
    in_maps = make_in_maps(x, qkv_w, qkv_b, proj_w)
    res = run_bass_kernel_spmd(nc, in_maps, core_ids=list(range(8)))
    return unshard(res.results, qkv_b, proj_w, proj_b)
